# revision 16
# baseline (speedup 1.0000x reference)
"""Trainium2 Bass kernel for nn_MultiHeadSSAN: banded Q/K (prefix-sum windows
along feature_len) + multi-head self-attention, sharded over the feature_len
(L) axis across 8 NeuronCores.

Per-core plan (core k owns n in [CH*k, CH*(k+1))):
  Band:  Q[s,t,e] = x + (C1[t-1]-C1[t-n1]) + (C2[min(t+n2-1,L-1)]-C2[t]) with
         C1=cumsum(x*a), C2=cumsum(x*c) along L.  Decomposed into per-chunk
         segmented scans (gated tensor_tensor_scan) on own chunk + one partner
         chunk (k -/+ OFF, host-prepared with sign/shift so the SPMD program is
         uniform), plus chunk-total boundary constants exchanged with an
         AllGather and folded in via the projection (B_proj identity-matmul).
  MHA:   per n: q/k projections + scores in fp32 on the PE; V-path in fp16.
         Softmax uses a transposed-scores layout so attn@V needs no transpose;
         the row max is subtracted via a rank-1 (K=1) matmul into PSUM.
"""
import math
import numpy as np

import concourse.bass as bass
import concourse.bacc as bacc
import concourse.mybir as mybir
import concourse.tile as tile
from concourse.bass_utils import run_bass_kernel_spmd

F32 = mybir.dt.float32
F32R = mybir.dt.float32r
BF16 = mybir.dt.bfloat16
F16 = mybir.dt.float16
ALU = mybir.AluOpType
ACTF = mybir.ActivationFunctionType
AX = mybir.AxisListType


class Cfg:
    def __init__(self, S=256, L=512, E=512, H=4, NC=8, OFF=4, SB=8,
                 logits_dtype="fp32", v_dtype="fp16"):
        self.S, self.L, self.E, self.H, self.NC = S, L, E, H, NC
        self.CH = L // NC              # L-chunk per core
        self.OFF = OFF                 # partner offset = n1 // CH
        self.n1 = self.n2 = OFF * self.CH
        self.HD = E // H
        assert self.HD == 128, "head dim must be 128"
        assert E % 128 == 0
        self.EB = E // 128             # e partition blocks
        self.SB = SB                   # band s-sub size
        assert S % SB == 0
        self.NST = (S + 127) // 128    # s tiles of <=128 in phase D
        self.STW = min(128, S)         # s tile width
        self.logits_dtype = logits_dtype
        self.v_dtype = v_dtype

    def key(self):
        return (self.S, self.L, self.E, self.H, self.NC, self.OFF, self.SB,
                self.logits_dtype, self.v_dtype)


def _mmcast(ap, dt):
    return ap.bitcast(dt) if dt is not None else ap


def build_nc(cfg: Cfg) -> bass.Bass:
    S, L, E, H, NC = cfg.S, cfg.L, cfg.E, cfg.H, cfg.NC
    CH, EB, SB, HD = cfg.CH, cfg.EB, cfg.SB, cfg.HD
    NSS = S // SB                      # band s-sub count
    BW = SB * CH                       # band tile free width
    NST, STW = cfg.NST, cfg.STW
    LDT = F32R if cfg.logits_dtype == "fp32r" else None   # None -> fp32 native
    VDT = F16 if cfg.v_dtype == "fp16" else None

    nc = bacc.Bacc(None)
    # ---- parameters
    xow = nc.declare_dram_parameter("xow", [S, CH, E], F32, isOutput=False)
    xp = nc.declare_dram_parameter("xp", [S, CH, E], F32, isOutput=False)
    wband = nc.declare_dram_parameter("wband", [6, CH, E], F32, isOutput=False)
    gate_in = nc.declare_dram_parameter("gate_in", [128, BW], F32, isOutput=False)
    coef = nc.declare_dram_parameter("coef", [128, 2 * NC], F32, isOutput=False)
    wq = nc.declare_dram_parameter("wq", [E, E], F32, isOutput=False)
    wk = nc.declare_dram_parameter("wk", [E, E], F32, isOutput=False)
    wv = nc.declare_dram_parameter("wv", [E, E], F16 if VDT else F32, isOutput=False)
    wo = nc.declare_dram_parameter("wo", [E, E], F16 if VDT else F32, isOutput=False)
    biasr = nc.declare_dram_parameter("biasr", [4, E], F32, isOutput=False)
    biasc = nc.declare_dram_parameter("biasc", [E, 4], F32, isOutput=False)
    ident_in = nc.declare_dram_parameter("ident_in", [128, 128], F32, isOutput=False)
    out = nc.declare_dram_parameter("out", [S, CH, E], F32, isOutput=True)

    # ---- internal DRAM
    qdram = nc.dram_tensor("qdram", [S, CH, E], F32)
    kdram = nc.dram_tensor("kdram", [S, CH, E], F32)
    tin = nc.dram_tensor("tin", [4, S, E], F32)
    tout = nc.dram_tensor("tout", [4 * NC, S, E], F32, addr_space="Shared")

    with tile.TileContext(nc) as tc:
        with (
            tc.tile_pool(name="const", bufs=1) as cpool,
            tc.tile_pool(name="band", bufs=2) as bpool,
            tc.tile_pool(name="scan", bufs=4) as spool,
            tc.tile_pool(name="bc", bufs=2) as bcpool,
            tc.tile_pool(name="dpool", bufs=2) as dpool,
            tc.tile_pool(name="evac", bufs=3) as epool,
            tc.tile_pool(name="ps_a", bufs=3, space="PSUM") as ps_a,
            tc.tile_pool(name="ps_b", bufs=3, space="PSUM") as ps_b,
            tc.tile_pool(name="ps_t", bufs=2, space="PSUM") as ps_t,
            tc.tile_pool(name="dbounce", bufs=4, space="DRAM") as dbpool,
        ):
            # ================= setup =================
            gate = cpool.tile([128, BW], F32, name="gate")
            nc.sync.dma_start(gate[:], gate_in[:, :])
            ident = cpool.tile([128, 128], F32, name="ident")
            nc.sync.dma_start(ident[:], ident_in[:, :])
            coef_sb = cpool.tile([128, 2 * NC], F32, name="coef_sb")
            nc.sync.dma_start(coef_sb[:], coef[:, :])
            biasrow = []
            for j in range(4):
                t = cpool.tile([1, E], F32, name=f"biasrow{j}")
                nc.sync.dma_start(t[:], biasr[j:j + 1, :])
                biasrow.append(t)
            biasc_sb = cpool.tile([E // 128 * 128, 4], F32, name="biasc_sb") \
                if False else cpool.tile([128, 4 * EB], F32, name="biasc_sb2")
            # biasc packed: biasc_sb2[:, 4*eb + j] = biasc[eb*128:(eb+1)*128, j]
            for eb in range(EB):
                nc.sync.dma_start(biasc_sb[:, 4 * eb:4 * (eb + 1)],
                                  biasc[eb * 128:(eb + 1) * 128, :])
            ones_row = cpool.tile([1, max(S, 128)], F32, name="ones_row")
            nc.vector.memset(ones_row[:], 1.0)

            wband_sb = []
            for kind in range(6):
                row = []
                for eb in range(EB):
                    t = cpool.tile([128, CH], F32, name=f"wband_{kind}_{eb}")
                    nc.sync.dma_start(
                        t[:], wband[kind, :, eb * 128:(eb + 1) * 128].transpose([1, 0]))
                    row.append(t)
                wband_sb.append(row)

            def load_w(dram, nm):
                tiles = []
                for eb in range(EB):
                    t = cpool.tile([128, E], F32, name=f"{nm}_{eb}")
                    nc.sync.dma_start(t[:], dram[eb * 128:(eb + 1) * 128, :])
                    tiles.append(t)
                return tiles

            wq_sb = load_w(wq, "wq")
            wk_sb = load_w(wk, "wk")
            wv_sb = []
            for eb in range(EB):
                t = cpool.tile([128, E], F16 if VDT else F32, name=f"wv_{eb}")
                nc.sync.dma_start(t[:], wv[eb * 128:(eb + 1) * 128, :])
                wv_sb.append(t)
            wo_v = []
            for eb in range(EB):
                t = cpool.tile([128, E], F16 if VDT else F32, name=f"wo_v_{eb}")
                nc.sync.dma_start(t[:], wo[eb * 128:(eb + 1) * 128, :])
                wo_v.append(t)

            # ================= band =================
            for eb in range(EB):
                er = slice(eb * 128, (eb + 1) * 128)
                for ss in range(NSS):
                    sr = slice(ss * SB, (ss + 1) * SB)
                    xb = bpool.tile([128, BW], F32, name="xb", tag="xb")
                    nc.sync.dma_start(xb[:], xow[sr, :, er].transpose([2, 0, 1]))
                    xpb = bpool.tile([128, BW], F32, name="xpb", tag="xpb")
                    nc.sync.dma_start(xpb[:], xp[sr, :, er].transpose([2, 0, 1]))

                    x3 = xb[:].rearrange("p (s l) -> p s l", l=CH)
                    xp3 = xpb[:].rearrange("p (s l) -> p s l", l=CH)

                    def prod(kind, src3, nm):
                        p = bpool.tile([128, BW], F32, name=nm, tag="prod", bufs=3)
                        wb = wband_sb[kind][eb][:].unsqueeze(1) \
                            .broadcast_to([128, SB, CH])
                        nc.vector.tensor_tensor(
                            p[:].rearrange("p (s l) -> p s l", l=CH),
                            src3, wb, op=ALU.mult)
                        return p

                    def scan(p, nm):
                        o = spool.tile([128, BW], F32, name=nm, tag="scan", bufs=8)
                        nc.vector.tensor_tensor_scan(
                            o[:], gate[:], p[:], 0.0,
                            op0=ALU.mult, op1=ALU.add)
                        return o

                    pa = prod(0, x3, "pa"); Ia = scan(pa, "Ia")
                    pc = prod(2, x3, "pc"); Ic = scan(pc, "Ic")
                    pp1 = prod(4, xp3, "pp1"); Ip1 = scan(pp1, "Ip1")
                    pb_ = prod(1, x3, "pb"); Ib = scan(pb_, "Ib")
                    pd = prod(3, x3, "pd"); Id = scan(pd, "Id")
                    pp2 = prod(5, xp3, "pp2"); Ip2 = scan(pp2, "Ip2")

                    # totals -> tin (kind, s, e)
                    for kind, I in ((0, Ia), (1, Ib), (2, Ic), (3, Id)):
                        tv = I[:].rearrange("p (s l) -> p s l", l=CH)[:, :, CH - 1]
                        nc.sync.dma_start(
                            tin[kind, sr, er].transpose([1, 0]), tv)

                    def assemble(I_fwd, I_sum_p, I_sum_own, qk, nm):
                        # out = x + E_fwd(view of I_fwd) + (I_sum_p - I_sum_own)
                        t1 = bpool.tile([128, BW], F32, name=f"t1{nm}", tag="t1")
                        t13 = t1[:].rearrange("p (s l) -> p s l", l=CH)
                        I3 = I_fwd[:].rearrange("p (s l) -> p s l", l=CH)
                        nc.vector.tensor_tensor(
                            t13[:, :, 1:CH], x3[:, :, 1:CH], I3[:, :, 0:CH - 1],
                            op=ALU.add)
                        nc.vector.tensor_copy(t13[:, :, 0:1], x3[:, :, 0:1])
                        ts = bpool.tile([128, BW], F32, name=f"ts{nm}", tag="ts")
                        nc.vector.tensor_tensor(
                            ts[:], I_sum_p[:], I_sum_own[:], op=ALU.subtract)
                        o = bpool.tile([128, BW], F32, name=f"o{nm}", tag="qk")
                        nc.vector.tensor_tensor(o[:], t1[:], ts[:], op=ALU.add)
                        dram = qdram if qk == "q" else kdram
                        nc.sync.dma_start(
                            dram[sr, :, er].transpose([2, 0, 1]), o[:])

                    assemble(Ia, Ip1, Ic, "q", "q")
                    assemble(Ib, Ip2, Id, "k", "k")

            # ================= totals exchange + B =================
            nc.gpsimd.collective_compute(
                "AllGather", ALU.bypass,
                replica_groups=[list(range(NC))],
                ins=[tin[:, :, :]], outs=[tout[:, :, :]],
            )
            # B_q/B_k per e-block: (128, S)
            Bq_eb, Bk_eb = [], []
            for eb in range(EB):
                er = slice(eb * 128, (eb + 1) * 128)
                for qk, kinds, dst in (("q", (0, 2), Bq_eb), ("k", (1, 3), Bk_eb)):
                    acc = cpool.tile([128, S], F32, name=f"B{qk}_{eb}")
                    nc.vector.memset(acc[:], 0.0)
                    for j in range(NC):
                        for ci, kind in enumerate(kinds):
                            tsl = bcpool.tile([128, S], F32, name="tsl", tag="tsl")
                            nc.sync.dma_start(
                                tsl[:],
                                tout[4 * j + kind, :, er].transpose([1, 0]))
                            nc.vector.scalar_tensor_tensor(
                                acc[:], tsl[:], coef_sb[:, ci * NC + j:ci * NC + j + 1],
                                acc[:], op0=ALU.mult, op1=ALU.add)
                    dst.append(acc)

            # B_proj (f-tiles) = W^T B + bias, kept in SBUF
            def bproj(w_sb, B_eb, bias_j, nm):
                tiles = []
                for fm in range(EB):
                    fr = slice(fm * 128, (fm + 1) * 128)
                    acc = ps_a.tile([128, S], F32, name=f"psB{nm}{fm}", tag="ps_mm")
                    for eb in range(EB):
                        nc.tensor.matmul(acc[:], w_sb[eb][:, fr], B_eb[eb][:],
                                         start=(eb == 0), stop=(eb == EB - 1))
                    o = cpool.tile([128, S], F32, name=f"B{nm}p_{fm}")
                    nc.vector.tensor_scalar_add(
                        o[:], acc[:], biasc_sb[:, 4 * fm + bias_j:4 * fm + bias_j + 1])
                    tiles.append(o)
                return tiles

            Bqp = bproj(wq_sb, Bq_eb, 0, "q")
            Bkp = bproj(wk_sb, Bk_eb, 1, "k")

            # ================= per-n attention =================
            for n in range(CH):
                qt, kt, xt = [], [], []
                for eb in range(EB):
                    er = slice(eb * 128, (eb + 1) * 128)
                    t = dpool.tile([128, S], F32, name=f"qt{eb}", tag=f"qt{eb}")
                    nc.sync.dma_start(t[:], qdram[:, n, er].transpose([1, 0]))
                    qt.append(t)
                    t = dpool.tile([128, S], F32, name=f"kt{eb}", tag=f"kt{eb}")
                    nc.sync.dma_start(t[:], kdram[:, n, er].transpose([1, 0]))
                    kt.append(t)
                    t = dpool.tile([128, S], F32, name=f"xt{eb}", tag=f"xt{eb}")
                    nc.sync.dma_start(t[:], xow[:, n, er].transpose([1, 0]))
                    xt.append(t)

                # q/k projections: (f, s) tiles
                def proj(w_sb, src, Bp, nm):
                    outt = []
                    for fm in range(EB):
                        fr = slice(fm * 128, (fm + 1) * 128)
                        acc = ps_a.tile([128, S], F32, name=f"ps{nm}{fm}", tag="ps_mm")
                        for eb in range(EB):
                            nc.tensor.matmul(
                                acc[:], _mmcast(w_sb[eb][:, fr], LDT),
                                _mmcast(src[eb][:], LDT),
                                start=(eb == 0), stop=False)
                        nc.tensor.matmul(acc[:], ident[:], Bp[fm][:],
                                         start=False, stop=True)
                        o = epool.tile([128, S], F32, name=f"{nm}_{fm}", tag="qkp", bufs=2 * EB + 4)
                        nc.scalar.activation(o[:], acc[:], ACTF.Copy)
                        outt.append(o)
                    return outt

                qp = proj(wq_sb, qt, Bqp, "qp")
                kp = proj(wk_sb, kt, Bkp, "kp")

                # v projection: (t, f) tiles [t = S axis]
                if VDT is not None:
                    xth = []
                    for eb in range(EB):
                        t = dpool.tile([128, S], F16, name=f"xth{eb}", tag=f"xth{eb}")
                        nc.vector.tensor_copy(t[:], xt[eb][:])
                        xth.append(t)
                else:
                    xth = xt
                vp = []
                for st in range(NST):
                    scols = slice(st * 128, st * 128 + STW)
                    acc = ps_a.tile([STW, E], F32, name=f"psv{st}", tag="ps_mm")
                    for eb in range(EB):
                        nc.tensor.matmul(acc[:], xth[eb][:, scols], wv_sb[eb][:],
                                         start=(eb == 0), stop=False)
                    nc.tensor.matmul(acc[:], ones_row[:1, :STW], biasrow[2][:1, :],
                                     start=False, stop=True)
                    o = epool.tile([STW, E], F16 if VDT else F32,
                                   name=f"vp_{st}", tag="vp", bufs=NST + 2)
                    nc.scalar.activation(o[:], acc[:], ACTF.Copy)
                    vp.append(o)

                # shift scores (s, t): max per (h, s)
                # shift scores (s, t) orientation -> per-(s,h) negated
                # log-sum-exp = -(max + log den); subtracting it inside the
                # transposed-score PSUM makes exp() emit normalized attn.
                lserow = []
                for st in range(NST):
                    scols = slice(st * 128, st * 128 + STW)
                    nmax_c = epool.tile([STW, H], F32, name=f"nmaxc{st}", tag="nmaxc", bufs=NST + 1)
                    den_c = epool.tile([STW, H], F32, name=f"denc{st}", tag="denc", bufs=NST + 1)
                    for h in range(H):
                        accs = ps_b.tile([STW, S], F32, name=f"pssh{st}{h}", tag="ps_sc")
                        nc.tensor.matmul(accs[:], _mmcast(qp[h][:, scols], LDT),
                                         _mmcast(kp[h][:], LDT),
                                         start=True, stop=True)
                        nc.vector.tensor_reduce(
                            nmax_c[:, h:h + 1], accs[:], axis=AX.X,
                            op=ALU.max, negate=True)
                        scr = epool.tile([STW, S], F16, name="escr", tag="escr")
                        nc.scalar.activation(
                            scr[:], accs[:], ACTF.Exp,
                            bias=nmax_c[:, h:h + 1], scale=1.0,
                            accum_out=den_c[:, h:h + 1])
                    ln_c = epool.tile([STW, H], F32, name=f"lnc{st}", tag="lnc", bufs=NST + 1)
                    nc.scalar.activation(ln_c[:], den_c[:], ACTF.Ln)
                    lse_c = epool.tile([STW, H], F32, name=f"lsec{st}", tag="lsec", bufs=NST + 1)
                    nc.vector.tensor_tensor(lse_c[:], nmax_c[:], ln_c[:],
                                            op=ALU.subtract)  # -(max) - ln(den)
                    # partition->free rearrange via DRAM bounce:
                    # row[0, s*H + h] = col[s, h]
                    bnc = dbpool.tile([STW, H], F32, name=f"lsebnc{st}", tag="lsebnc")
                    nc.sync.dma_start(bnc[:], lse_c[:])
                    lr = epool.tile([1, STW * H], F32, name=f"lserow{st}", tag="lserow", bufs=NST + 1)
                    nc.sync.dma_start(lr[:], bnc[:].rearrange("s h -> (s h)").unsqueeze(0))
                    lserow.append(lr)

                def hrow(rows, st, h):
                    # strided (1, STW) view: elements [h], [H+h], [2H+h], ...
                    return rows[st][:].rearrange("o (s h) -> o s h", h=H)[:, :, h]

                # scores^T - lse + exp -> normalized attn^T (t, s), per head
                PT = []
                for h in range(H):
                    row = []
                    for tt in range(NST):
                        tcols = slice(tt * 128, tt * 128 + STW)
                        acc = ps_b.tile([STW, S], F32, name=f"psT{h}{tt}", tag="ps_sc")
                        nc.tensor.matmul(acc[:], _mmcast(kp[h][:, tcols], LDT),
                                         _mmcast(qp[h][:], LDT),
                                         start=True, stop=False)
                        for st in range(NST):
                            scols = slice(st * 128, st * 128 + STW)
                            nc.tensor.matmul(
                                acc[:, scols], ones_row[:1, :STW],
                                hrow(lserow, st, h),
                                start=False, stop=(st == NST - 1))
                        p = epool.tile([STW, S], F16 if VDT else F32,
                                       name=f"PT{h}{tt}", tag="PT", bufs=H * NST + 2)
                        nc.scalar.activation(p[:], acc[:], ACTF.Exp)
                        row.append(p)
                    PT.append(row)

                # attn @ V -> o^T (hd, s) per head
                osc = []
                for h in range(H):
                    hr = slice(h * HD, (h + 1) * HD)
                    acc = ps_t.tile([HD, S], F32, name=f"pso{h}", tag="ps_oo")
                    for tt in range(NST):
                        tcols = slice(tt * 128, tt * 128 + STW)
                        nc.tensor.matmul(acc[:], vp[tt][:, hr], PT[h][tt][:],
                                         start=(tt == 0), stop=(tt == NST - 1))
                    o = epool.tile([HD, S], F16 if VDT else F32,
                                   name=f"osc{h}", tag="osc", bufs=H + 1)
                    nc.scalar.activation(o[:], acc[:], ACTF.Copy)
                    osc.append(o)

                # out projection: (g, s) tiles
                for gm in range(EB):
                    gr = slice(gm * 128, (gm + 1) * 128)
                    acc = ps_a.tile([128, S], F32, name=f"psout{gm}", tag="ps_mm")
                    for fm in range(EB):
                        nc.tensor.matmul(acc[:], wo_v[fm][:, gr], osc[fm][:],
                                         start=(fm == 0), stop=False)
                    nc.tensor.matmul(acc[:], biasrow[3][:1, gr], ones_row[:1, :S],
                                     start=False, stop=True)
                    o = epool.tile([128, S], F32, name=f"oo{gm}", tag="oo")
                    nc.scalar.activation(o[:], acc[:], ACTF.Copy)
                    nc.sync.dma_start(out[:, n, gr].transpose([1, 0]), o[:])

    nc.finalize()
    return nc


# ============================================================
# host side
# ============================================================

def prep_inputs(cfg: Cfg, x, a, b, c, d, in_proj_w, in_proj_b, out_w, out_b):
    S, L, E, NC, CH, OFF = cfg.S, cfg.L, cfg.E, cfg.NC, cfg.CH, cfg.OFF
    f32 = np.float32
    x = np.asarray(x, f32)
    hd = cfg.HD
    scl = 1.0 / math.sqrt(hd)
    wq = np.ascontiguousarray(in_proj_w[:E].T * scl).astype(f32)
    wk = np.ascontiguousarray(in_proj_w[E:2 * E].T).astype(f32)
    wv = np.ascontiguousarray(in_proj_w[2 * E:].T).astype(
        np.float16 if cfg.v_dtype == 'fp16' else f32)
    wo = np.ascontiguousarray(out_w.T).astype(
        np.float16 if cfg.v_dtype == 'fp16' else f32)
    bq = in_proj_b[:E] * scl
    bk = in_proj_b[E:2 * E]
    bv = in_proj_b[2 * E:]
    bo = out_b
    biasr = np.stack([bq, bk, bv, bo]).astype(f32)
    biasc = np.ascontiguousarray(biasr.T).astype(f32)
    ident = np.eye(128, dtype=f32)

    gate = np.ones((128, cfg.SB * CH), f32)
    gate[:, ::CH] = 0.0

    in_maps = []
    for k in range(NC):
        chk = slice(CH * k, CH * (k + 1))
        xow = x[:, chk, :]
        if k >= OFF:
            pf = slice(CH * (k - OFF), CH * (k - OFF + 1))
            xp = x[:, pf, :]
            w1 = -a[pf].astype(f32)
            w2 = -b[pf].astype(f32)
        else:
            st = CH * (k + OFF) - 1
            xp = x[:, st:st + CH, :].copy()
            xp[:, 0, :] = 0.0
            w1 = np.zeros((CH, E), f32)
            w1[1:] = c[st + 1:st + CH]
            w2 = np.zeros((CH, E), f32)
            w2[1:] = d[st + 1:st + CH]
        wband = np.stack([a[chk], b[chk], c[chk], d[chk], w1, w2]).astype(f32)
        coefA = np.zeros(NC, f32)
        coefA[max(0, k - OFF):k] = 1.0
        coefC = np.zeros(NC, f32)
        coefC[k:min(k + OFF - 1, NC - 1) + 1] = 1.0
        coef = np.broadcast_to(
            np.concatenate([coefA, coefC])[None, :], (128, 2 * NC)).copy()
        in_maps.append(dict(
            xow=np.ascontiguousarray(xow), xp=np.ascontiguousarray(xp),
            wband=wband, gate_in=gate, coef=coef,
            wq=wq, wk=wk, wv=wv, wo=wo, biasr=biasr, biasc=biasc,
            ident_in=ident,
        ))
    return in_maps


_CACHE = {}


def run(cfg: Cfg, inputs, core_ids=None, **kw):
    key = cfg.key()
    if key not in _CACHE:
        _CACHE[key] = build_nc(cfg)
    nc = _CACHE[key]
    in_maps = prep_inputs(
        cfg, inputs["x"], inputs["a"], inputs["b"], inputs["c"], inputs["d"],
        inputs["in_proj_w"], inputs["in_proj_b"], inputs["out_w"], inputs["out_b"])
    res = run_bass_kernel_spmd(nc, in_maps, core_ids or list(range(cfg.NC)), **kw)
    S, L, E, CH = cfg.S, cfg.L, cfg.E, cfg.CH
    full = np.empty((S, L, E), np.float32)
    for k in range(cfg.NC):
        full[:, CH * k:CH * (k + 1), :] = res.results[k]["out"]
    return full, res


def kernel(**inputs) -> np.ndarray:
    assert int(inputs["n1"]) == 256 and int(inputs["n2"]) == 256
    cfg = Cfg()
    out, _ = run(cfg, inputs)
    return out
